# revision 9
# baseline (speedup 1.0000x reference)
"""GCN layer (gather-scatter message passing) on 8 Trainium2 NeuronCores.

Strategy (dest-node sharding, per the sharding hint):
  out = segment_sum(w_e * (xW+b)[col_e] by row_e)
      = (A @ x) @ W + deg * b          with deg = A @ 1
so the big gather/scatter runs on raw x and the dense transform is applied
once per destination block afterwards.

Per core (SPMD, identical program):
  - 98 dest blocks of 128 nodes; host assigns nodes to blocks, balancing
    per-(block, source-chunk) edge counts under a hard cap.
  - x (bf16) is gathered per edge straight from DRAM via gpsimd dma_gather
    (int16 indices -> 4 source chunks of 25088 rows).
  - Per 128-edge tile: PE accumulates psum1[ch,dest] += X_t^T @ M_t and
    psum_deg[1,dest] += ones^T @ M_t, where M_t[e,d] = w_e*(dest[e]==d) is
    the host-prepared one-hot scatter operator (bf16, streamed from DRAM).
  - Per block: out_block = psum1^T @ W + deg^T @ b (fp32), DMA out.
Host gathers the 8 outputs and undoes the node permutation.
"""
import sys
sys.path.insert(0, "/opt/trn_rl_repo")
import numpy as np

N_NODES = 100000
N_EDGES = 625000
D = 128
NCORES = 8
BLK = 128
NB = 98                      # dest blocks per core
NPC = NB * BLK               # 12544 nodes per core
NBINS = NCORES * NB          # 784
NCHUNK = 4
CH_ROWS = 25088              # source chunk rows (int16-addressable)
XPAD = NCHUNK * CH_ROWS      # 100352
GB = 14                      # blocks per gather group
NG = NB // GB                # 7 groups
CAP_BASE = 2                 # tiles per (block, chunk) unless data demands more

_cache = {}
_last = {}


def _build_program(tbc):
    from concourse import bass, mybir
    import concourse.tile as tile
    from concourse import library_config

    tiles_per_block = NCHUNK * tbc
    t_total = NB * tiles_per_block
    call_idxs = GB * tbc * BLK
    idx_cols = call_idxs // 16
    ncalls = NCHUNK * NG
    slots = GB * tbc

    nc = bass.Bass(num_swdge_queues=4)
    x_d = nc.declare_dram_parameter("x", [XPAD, D], mybir.dt.bfloat16, isOutput=False)
    idx_d = nc.declare_dram_parameter("idx", [128, ncalls * idx_cols], mybir.dt.int16, isOutput=False)
    m_d = nc.declare_dram_parameter("m", [128, t_total * BLK], mybir.dt.bfloat16, isOutput=False)
    W_d = nc.declare_dram_parameter("W", [128, 128], mybir.dt.float32, isOutput=False)
    b_d = nc.declare_dram_parameter("b", [1, 128], mybir.dt.float32, isOutput=False)
    ones_d = nc.declare_dram_parameter("ones", [128, 1], mybir.dt.bfloat16, isOutput=False)
    out_d = nc.declare_dram_parameter("out", [NPC, D], mybir.dt.float32, isOutput=True)

    with tile.TileContext(nc) as tc:
        with (
            tc.tile_pool(name="persist", bufs=1) as persist,
            tc.tile_pool(name="stage", bufs=8) as stage,
            tc.tile_pool(name="mstage", bufs=6) as mstage,
            tc.tile_pool(name="sbout", bufs=4) as sbout,
            tc.tile_pool(name="psum1", bufs=2, space="PSUM") as psum1p,
            tc.tile_pool(name="psumd", bufs=2, space="PSUM") as psumdp,
            tc.tile_pool(name="psum2", bufs=2, space="PSUM") as psum2p,
        ):
            with tc.tile_critical():
                nc.gpsimd.load_library(library_config.mlp)
            idx_t = persist.tile([128, ncalls * idx_cols], mybir.dt.int16)
            nc.sync.dma_start(out=idx_t[:], in_=idx_d[:])
            W_t = persist.tile([128, 128], mybir.dt.float32)
            nc.sync.dma_start(out=W_t[:], in_=W_d[:])
            b_t = persist.tile([1, 128], mybir.dt.float32)
            nc.sync.dma_start(out=b_t[:], in_=b_d[:])
            ones_t = persist.tile([128, 1], mybir.dt.bfloat16)
            nc.sync.dma_start(out=ones_t[:], in_=ones_d[:])

            for g in range(NG):
                stages = []
                for c in range(NCHUNK):
                    call_index = g * NCHUNK + c
                    st = stage.tile([128, slots, 128], mybir.dt.bfloat16)
                    nc.gpsimd.dma_gather(
                        out_ap=st[:],
                        in_ap=x_d[c * CH_ROWS:(c + 1) * CH_ROWS, :],
                        idxs_ap=idx_t[:, call_index * idx_cols:(call_index + 1) * idx_cols],
                        num_idxs=call_idxs,
                        num_idxs_reg=call_idxs,
                        elem_size=D,
                        single_packet=False,
                        queue_num=1 + call_index % 3,
                    )
                    stages.append(st)
                for bloc in range(GB):
                    b = g * GB + bloc
                    mt = mstage.tile([128, tiles_per_block, BLK], mybir.dt.bfloat16)
                    nc.sync.dma_start(
                        out=mt[:],
                        in_=m_d[:, b * tiles_per_block * BLK:(b + 1) * tiles_per_block * BLK],
                    )
                    ps1 = psum1p.tile([128, 128], mybir.dt.float32, space="PSUM")
                    psd = psumdp.tile([1, 128], mybir.dt.float32, space="PSUM")
                    ti = 0
                    for c in range(NCHUNK):
                        for k in range(tbc):
                            slot = bloc * tbc + k
                            xs = stages[c][:, slot, :]
                            ms = mt[:, c * tbc + k, :]
                            first, last = ti == 0, ti == tiles_per_block - 1
                            nc.tensor.matmul(ps1[:], lhsT=xs, rhs=ms, start=first, stop=last)
                            nc.tensor.matmul(psd[:], lhsT=ones_t[:], rhs=ms, start=first, stop=last)
                            ti += 1
                    axT = sbout.tile([128, 128], mybir.dt.float32)
                    nc.scalar.mul(axT[:], ps1[:], 1.0)
                    deg = sbout.tile([1, 128], mybir.dt.float32)
                    nc.scalar.mul(deg[:], psd[:], 1.0)
                    ps2 = psum2p.tile([128, 128], mybir.dt.float32, space="PSUM")
                    nc.tensor.matmul(ps2[:], lhsT=axT[:], rhs=W_t[:], start=True, stop=False)
                    nc.tensor.matmul(ps2[:], lhsT=deg[:], rhs=b_t[:], start=False, stop=True)
                    ob = sbout.tile([128, 128], mybir.dt.float32)
                    nc.scalar.mul(ob[:], ps2[:], 1.0)
                    nc.sync.dma_start(out=out_d[b * BLK:(b + 1) * BLK, :], in_=ob[:])

    mybir.codegen_inst_isa_subclasses(nc)
    _fix_multiwait(nc)
    return nc


def _fix_multiwait(nc):
    """This walrus build supports ONE sync-wait per instruction; split any
    instruction carrying more onto same-engine wait-carrier nops."""
    from concourse import mybir
    ctr = 0
    for fn in nc.m.functions:
        for bb in fn.blocks:
            live = bb.instructions
            snap = list(live)
            pos = 0
            for inst in snap:
                si = inst.sync_info
                if si is not None and si.on_wait is not None and len(si.on_wait) > 1:
                    waits = list(si.on_wait)
                    si.on_wait = [waits[-1]]
                    for w in waits[:-1]:
                        n = mybir.InstNoOp(name=f"mwsplit{ctr}", ins=[], outs=[])
                        ctr += 1
                        n.engine = inst.engine
                        n.sync_info = type(si)(on_wait=[w], on_update=[])
                        live.insert(pos, n)
                        pos += 1
                pos += 1


def _prepare(x, edge_index, edge_weight):
    """Host-side sharding: node->bin assignment + per-core tile layout."""
    dest = np.asarray(edge_index[0], dtype=np.int64)
    src = np.asarray(edge_index[1], dtype=np.int64)
    w = np.asarray(edge_weight, dtype=np.float32)

    deg = np.bincount(dest, minlength=N_NODES)
    order = np.argsort(-deg, kind="stable")
    bin_of_node = np.empty(N_NODES, dtype=np.int64)
    bin_of_node[order] = np.arange(N_NODES, dtype=np.int64) % NBINS

    echunk = src // CH_ROWS
    cap = CAP_BASE * BLK

    # repair: enforce per-(bin, chunk) edge count <= cap by moving nodes
    for _ in range(200):
        ekey = bin_of_node[dest] * NCHUNK + echunk
        counts = np.bincount(ekey, minlength=NBINS * NCHUNK).reshape(NBINS, NCHUNK)
        bin_sizes = np.bincount(bin_of_node, minlength=NBINS)
        over = np.argwhere(counts > cap)
        if len(over) == 0:
            break
        bidx, c = over[0]
        nodes_in_bin = np.where(bin_of_node == bidx)[0]
        sel = (bin_of_node[dest] == bidx) & (echunk == c)
        nd = np.bincount(dest[sel], minlength=N_NODES)[nodes_in_bin]
        mover = nodes_in_bin[np.argmax(nd)]
        room = (bin_sizes < BLK)
        cand = np.where(room)[0]
        if len(cand) == 0:
            cand = np.arange(NBINS)
        tgt = cand[np.argmin(counts[cand, c])]
        bin_of_node[mover] = tgt

    ekey = bin_of_node[dest] * NCHUNK + echunk
    counts = np.bincount(ekey, minlength=NBINS * NCHUNK).reshape(NBINS, NCHUNK)
    tbc = max(CAP_BASE, int(np.ceil(counts.max() / BLK)))

    # position of each node within its block
    nodesort = np.argsort(bin_of_node, kind="stable")
    bin_sizes = np.bincount(bin_of_node, minlength=NBINS)
    starts = np.zeros(NBINS + 1, dtype=np.int64)
    np.cumsum(bin_sizes, out=starts[1:])
    pos_in_block = np.empty(N_NODES, dtype=np.int64)
    pos_in_block[nodesort] = np.arange(N_NODES) - starts[bin_of_node[nodesort]]
    assert pos_in_block.max() < BLK

    slot_of_node = bin_of_node * BLK + pos_in_block
    ebin = bin_of_node[dest]
    ekey = ebin * NCHUNK + echunk
    edge_order = np.argsort(ekey, kind="stable")
    ekey_s = ekey[edge_order]
    key_counts = np.bincount(ekey_s, minlength=NBINS * NCHUNK)
    key_starts = np.zeros(NBINS * NCHUNK + 1, dtype=np.int64)
    np.cumsum(key_counts, out=key_starts[1:])
    lane = np.arange(N_EDGES) - key_starts[ekey_s]

    es = {
        "src": src[edge_order], "w": w[edge_order],
        "dlocal": pos_in_block[dest[edge_order]],
        "bin": ebin[edge_order], "chunk": echunk[edge_order], "lane": lane,
    }
    return es, slot_of_node, tbc


def _build_core_arrays(es, core, tbc):
    import ml_dtypes
    tiles_per_block = NCHUNK * tbc
    t_total = NB * tiles_per_block
    call_idxs = GB * tbc * BLK
    idx_cols = call_idxs // 16
    ncalls = NCHUNK * NG

    sel = (es["bin"] >= core * NB) & (es["bin"] < (core + 1) * NB)
    b = es["bin"][sel] - core * NB
    c = es["chunk"][sel]
    lane = es["lane"][sel]
    srcl = (es["src"][sel] - c * CH_ROWS).astype(np.int16)
    dl = es["dlocal"][sel]
    wv = es["w"][sel]

    k = lane // BLK
    p = lane % BLK
    t_loc = b * tiles_per_block + c * tbc + k

    # host-built one-hot scatter operator: m[p, t_loc*BLK + dl] = w
    m_arr = np.zeros((128, t_total * BLK), ml_dtypes.bfloat16)
    m_arr[p, t_loc * BLK + dl] = wv

    # gather call layout: call (g, c) at call_index = g*NCHUNK+c
    g = b // GB
    call_index = g * NCHUNK + c
    i_call = (b % GB) * (tbc * BLK) + lane
    flat_idx = np.zeros(ncalls * call_idxs, np.int16)
    flat_idx[call_index * call_idxs + i_call] = srcl
    idx_arr = np.empty((128, ncalls * idx_cols), np.int16)
    wrapped = (
        flat_idx.reshape(ncalls, idx_cols, 16)
        .transpose(2, 0, 1)
        .reshape(16, ncalls * idx_cols)
    )
    for gg in range(8):
        idx_arr[gg * 16:(gg + 1) * 16, :] = wrapped
    return idx_arr, m_arr


def kernel(x, edge_index, edge_weight, W, b):
    import ml_dtypes
    from concourse.bass_utils import run_bass_kernel_spmd

    x = np.asarray(x, dtype=np.float32)
    W_np = np.asarray(W, dtype=np.float32)
    b_np = np.asarray(b, dtype=np.float32).reshape(1, D)

    es, slot_of_node, tbc = _prepare(x, edge_index, edge_weight)

    if tbc not in _cache:
        _cache[tbc] = _build_program(tbc)
    nc = _cache[tbc]

    x_pad = np.zeros((XPAD, D), ml_dtypes.bfloat16)
    x_pad[:N_NODES] = x.astype(ml_dtypes.bfloat16)
    ones = np.ones((128, 1), ml_dtypes.bfloat16)

    in_maps = []
    for core in range(NCORES):
        idx_arr, m_arr = _build_core_arrays(es, core, tbc)
        in_maps.append({
            "x": x_pad, "idx": idx_arr, "m": m_arr,
            "W": W_np, "b": b_np, "ones": ones,
        })

    _last["nc"] = nc
    _last["in_maps"] = in_maps
    res = run_bass_kernel_spmd(nc, in_maps, list(range(NCORES)))

    full = np.concatenate([res.results[c]["out"] for c in range(NCORES)], axis=0)
    out = np.empty((N_NODES, D), np.float32)
    out[:] = full[slot_of_node]
    return out


# revision 11
# speedup vs baseline: 1.2177x; 1.2177x over previous
"""GCN layer (gather-scatter message passing) on 8 Trainium2 NeuronCores.

Strategy (dest-node sharding, per the sharding hint):
  out = segment_sum(w_e * (xW+b)[col_e] by row_e)
      = (A @ x) @ W + deg * b          with deg = A @ 1
so the big gather/scatter runs on raw x and the dense transform is applied
once per destination block afterwards.

Per core (SPMD, identical program):
  - 98 dest blocks of 128 nodes; host assigns nodes to blocks, balancing
    per-(block, source-chunk) edge counts under a hard cap.
  - x (bf16) is gathered per edge straight from DRAM via gpsimd dma_gather
    (int16 indices -> 4 source chunks of 25088 rows).
  - Per 128-edge tile: PE accumulates psum1[ch,dest] += X_t^T @ M_t and
    psum_deg[1,dest] += ones^T @ M_t, where M_t[e,d] = w_e*(dest[e]==d) is
    the host-prepared one-hot scatter operator (bf16, streamed from DRAM).
  - Per block: out_block = psum1^T @ W + deg^T @ b (fp32), DMA out.
Host gathers the 8 outputs and undoes the node permutation.
"""
import sys
sys.path.insert(0, "/opt/trn_rl_repo")
import numpy as np

N_NODES = 100000
N_EDGES = 625000
D = 128
NCORES = 8
BLK = 128
NB = 98                      # dest blocks per core
NPC = NB * BLK               # 12544 nodes per core
NBINS = NCORES * NB          # 784
NCHUNK = 4
CH_ROWS = 25088              # source chunk rows (int16-addressable)
XPAD = NCHUNK * CH_ROWS      # 100352
GB = 14                      # blocks per gather group
NG = NB // GB                # 7 groups
CAP_BASE = 2                 # tiles per (block, chunk) unless data demands more

_cache = {}
_last = {}


def _build_program(tbc):
    from concourse import bass, mybir
    import concourse.tile as tile
    from concourse import library_config

    tiles_per_block = NCHUNK * tbc
    t_total = NB * tiles_per_block
    call_idxs = GB * tbc * BLK
    idx_cols = call_idxs // 16
    ncalls = NCHUNK * NG
    slots = GB * tbc

    nc = bass.Bass(num_swdge_queues=4)
    x_d = nc.declare_dram_parameter("x", [XPAD, D], mybir.dt.bfloat16, isOutput=False)
    idx_d = nc.declare_dram_parameter("idx", [128, ncalls * idx_cols], mybir.dt.int16, isOutput=False)
    m_d = nc.declare_dram_parameter("m", [128, t_total * BLK], mybir.dt.bfloat16, isOutput=False)
    W_d = nc.declare_dram_parameter("W", [128, 128], mybir.dt.float32, isOutput=False)
    b_d = nc.declare_dram_parameter("b", [1, 128], mybir.dt.float32, isOutput=False)
    ones_d = nc.declare_dram_parameter("ones", [128, 1], mybir.dt.bfloat16, isOutput=False)
    out_d = nc.declare_dram_parameter("out", [NPC, D], mybir.dt.float32, isOutput=True)

    with tile.TileContext(nc) as tc:
        with (
            tc.tile_pool(name="persist", bufs=1) as persist,
            tc.tile_pool(name="stage", bufs=8) as stage,
            tc.tile_pool(name="mstage", bufs=6) as mstage,
            tc.tile_pool(name="sbout", bufs=4) as sbout,
            tc.tile_pool(name="psum1", bufs=2, space="PSUM") as psum1p,
            tc.tile_pool(name="psumd", bufs=2, space="PSUM") as psumdp,
            tc.tile_pool(name="psum2", bufs=2, space="PSUM") as psum2p,
        ):
            with tc.tile_critical():
                nc.gpsimd.load_library(library_config.mlp)
            idx_t = persist.tile([128, ncalls * idx_cols], mybir.dt.int16)
            nc.sync.dma_start(out=idx_t[:], in_=idx_d[:])
            W_t = persist.tile([128, 128], mybir.dt.float32)
            nc.sync.dma_start(out=W_t[:], in_=W_d[:])
            b_t = persist.tile([1, 128], mybir.dt.float32)
            nc.sync.dma_start(out=b_t[:], in_=b_d[:])
            ones_t = persist.tile([128, 1], mybir.dt.bfloat16)
            nc.sync.dma_start(out=ones_t[:], in_=ones_d[:])

            for g in range(NG):
                stages = []
                for c in range(NCHUNK):
                    call_index = g * NCHUNK + c
                    st = stage.tile([128, slots, 128], mybir.dt.bfloat16)
                    nc.gpsimd.dma_gather(
                        out_ap=st[:],
                        in_ap=x_d[c * CH_ROWS:(c + 1) * CH_ROWS, :],
                        idxs_ap=idx_t[:, call_index * idx_cols:(call_index + 1) * idx_cols],
                        num_idxs=call_idxs,
                        num_idxs_reg=call_idxs,
                        elem_size=D,
                        single_packet=False,
                        queue_num=call_index % 4,
                    )
                    stages.append(st)
                for bloc in range(GB):
                    b = g * GB + bloc
                    mt = mstage.tile([128, tiles_per_block, BLK], mybir.dt.bfloat16)
                    nc.sync.dma_start(
                        out=mt[:],
                        in_=m_d[:, b * tiles_per_block * BLK:(b + 1) * tiles_per_block * BLK],
                    )
                    ps1 = psum1p.tile([128, 128], mybir.dt.float32, space="PSUM")
                    psd = psumdp.tile([1, 512], mybir.dt.float32, space="PSUM")
                    ti = 0
                    for c in range(NCHUNK):
                        for k in range(tbc):
                            slot = bloc * tbc + k
                            xs = stages[c][:, slot, :]
                            ms = mt[:, c * tbc + k, :]
                            first, last = ti == 0, ti == tiles_per_block - 1
                            nc.tensor.matmul(ps1[:], lhsT=xs, rhs=ms, start=first, stop=last)
                            ti += 1
                    half = tiles_per_block // 2
                    nc.tensor.matmul(psd[:], lhsT=ones_t[:],
                                     rhs=mt[:, 0:half, :].rearrange("p a b -> p (a b)"),
                                     start=True, stop=False)
                    nc.tensor.matmul(psd[:], lhsT=ones_t[:],
                                     rhs=mt[:, half:tiles_per_block, :].rearrange("p a b -> p (a b)"),
                                     start=False, stop=True)
                    axT = sbout.tile([128, 128], mybir.dt.float32)
                    nc.scalar.mul(axT[:], ps1[:], 1.0)
                    deg = sbout.tile([1, 128], mybir.dt.float32)
                    nc.vector.tensor_reduce(
                        out=deg[:], in_=psd[:].rearrange("p (a b) -> p b a", a=half),
                        axis=mybir.AxisListType.X, op=mybir.AluOpType.add)
                    ps2 = psum2p.tile([128, 128], mybir.dt.float32, space="PSUM")
                    nc.tensor.matmul(ps2[:], lhsT=axT[:], rhs=W_t[:], start=True, stop=False)
                    nc.tensor.matmul(ps2[:], lhsT=deg[:], rhs=b_t[:], start=False, stop=True)
                    ob = sbout.tile([128, 128], mybir.dt.float32)
                    nc.scalar.mul(ob[:], ps2[:], 1.0)
                    nc.sync.dma_start(out=out_d[b * BLK:(b + 1) * BLK, :], in_=ob[:])

    mybir.codegen_inst_isa_subclasses(nc)
    _fix_multiwait(nc)
    return nc


def _fix_multiwait(nc):
    """This walrus build supports ONE sync-wait per instruction; split any
    instruction carrying more onto same-engine wait-carrier nops."""
    from concourse import mybir
    ctr = 0
    for fn in nc.m.functions:
        for bb in fn.blocks:
            live = bb.instructions
            snap = list(live)
            pos = 0
            for inst in snap:
                si = inst.sync_info
                if si is not None and si.on_wait is not None and len(si.on_wait) > 1:
                    waits = list(si.on_wait)
                    si.on_wait = [waits[-1]]
                    for w in waits[:-1]:
                        n = mybir.InstNoOp(name=f"mwsplit{ctr}", ins=[], outs=[])
                        ctr += 1
                        n.engine = inst.engine
                        n.sync_info = type(si)(on_wait=[w], on_update=[])
                        live.insert(pos, n)
                        pos += 1
                pos += 1


def _prepare(x, edge_index, edge_weight):
    """Host-side sharding: node->bin assignment + per-core tile layout."""
    dest = np.asarray(edge_index[0], dtype=np.int64)
    src = np.asarray(edge_index[1], dtype=np.int64)
    w = np.asarray(edge_weight, dtype=np.float32)

    deg = np.bincount(dest, minlength=N_NODES)
    order = np.argsort(-deg, kind="stable")
    bin_of_node = np.empty(N_NODES, dtype=np.int64)
    bin_of_node[order] = np.arange(N_NODES, dtype=np.int64) % NBINS

    echunk = src // CH_ROWS
    cap = CAP_BASE * BLK

    # repair: enforce per-(bin, chunk) edge count <= cap by moving nodes
    for _ in range(200):
        ekey = bin_of_node[dest] * NCHUNK + echunk
        counts = np.bincount(ekey, minlength=NBINS * NCHUNK).reshape(NBINS, NCHUNK)
        bin_sizes = np.bincount(bin_of_node, minlength=NBINS)
        over = np.argwhere(counts > cap)
        if len(over) == 0:
            break
        bidx, c = over[0]
        nodes_in_bin = np.where(bin_of_node == bidx)[0]
        sel = (bin_of_node[dest] == bidx) & (echunk == c)
        nd = np.bincount(dest[sel], minlength=N_NODES)[nodes_in_bin]
        mover = nodes_in_bin[np.argmax(nd)]
        room = (bin_sizes < BLK)
        cand = np.where(room)[0]
        if len(cand) == 0:
            cand = np.arange(NBINS)
        tgt = cand[np.argmin(counts[cand, c])]
        bin_of_node[mover] = tgt

    ekey = bin_of_node[dest] * NCHUNK + echunk
    counts = np.bincount(ekey, minlength=NBINS * NCHUNK).reshape(NBINS, NCHUNK)
    tbc = max(CAP_BASE, int(np.ceil(counts.max() / BLK)))

    # position of each node within its block
    nodesort = np.argsort(bin_of_node, kind="stable")
    bin_sizes = np.bincount(bin_of_node, minlength=NBINS)
    starts = np.zeros(NBINS + 1, dtype=np.int64)
    np.cumsum(bin_sizes, out=starts[1:])
    pos_in_block = np.empty(N_NODES, dtype=np.int64)
    pos_in_block[nodesort] = np.arange(N_NODES) - starts[bin_of_node[nodesort]]
    assert pos_in_block.max() < BLK

    slot_of_node = bin_of_node * BLK + pos_in_block
    ebin = bin_of_node[dest]
    ekey = ebin * NCHUNK + echunk
    edge_order = np.argsort(ekey, kind="stable")
    ekey_s = ekey[edge_order]
    key_counts = np.bincount(ekey_s, minlength=NBINS * NCHUNK)
    key_starts = np.zeros(NBINS * NCHUNK + 1, dtype=np.int64)
    np.cumsum(key_counts, out=key_starts[1:])
    lane = np.arange(N_EDGES) - key_starts[ekey_s]

    es = {
        "src": src[edge_order], "w": w[edge_order],
        "dlocal": pos_in_block[dest[edge_order]],
        "bin": ebin[edge_order], "chunk": echunk[edge_order], "lane": lane,
    }
    return es, slot_of_node, tbc


def _build_core_arrays(es, core, tbc):
    import ml_dtypes
    tiles_per_block = NCHUNK * tbc
    t_total = NB * tiles_per_block
    call_idxs = GB * tbc * BLK
    idx_cols = call_idxs // 16
    ncalls = NCHUNK * NG

    sel = (es["bin"] >= core * NB) & (es["bin"] < (core + 1) * NB)
    b = es["bin"][sel] - core * NB
    c = es["chunk"][sel]
    lane = es["lane"][sel]
    srcl = (es["src"][sel] - c * CH_ROWS).astype(np.int16)
    dl = es["dlocal"][sel]
    wv = es["w"][sel]

    k = lane // BLK
    p = lane % BLK
    t_loc = b * tiles_per_block + c * tbc + k

    # host-built one-hot scatter operator: m[p, t_loc*BLK + dl] = w
    m_arr = np.zeros((128, t_total * BLK), ml_dtypes.bfloat16)
    m_arr[p, t_loc * BLK + dl] = wv

    # gather call layout: call (g, c) at call_index = g*NCHUNK+c
    g = b // GB
    call_index = g * NCHUNK + c
    i_call = (b % GB) * (tbc * BLK) + lane
    flat_idx = np.zeros(ncalls * call_idxs, np.int16)
    flat_idx[call_index * call_idxs + i_call] = srcl
    idx_arr = np.empty((128, ncalls * idx_cols), np.int16)
    wrapped = (
        flat_idx.reshape(ncalls, idx_cols, 16)
        .transpose(2, 0, 1)
        .reshape(16, ncalls * idx_cols)
    )
    for gg in range(8):
        idx_arr[gg * 16:(gg + 1) * 16, :] = wrapped
    return idx_arr, m_arr


def kernel(x, edge_index, edge_weight, W, b):
    import ml_dtypes
    from concourse.bass_utils import run_bass_kernel_spmd

    x = np.asarray(x, dtype=np.float32)
    W_np = np.asarray(W, dtype=np.float32)
    b_np = np.asarray(b, dtype=np.float32).reshape(1, D)

    es, slot_of_node, tbc = _prepare(x, edge_index, edge_weight)

    if tbc not in _cache:
        _cache[tbc] = _build_program(tbc)
    nc = _cache[tbc]

    x_pad = np.zeros((XPAD, D), ml_dtypes.bfloat16)
    x_pad[:N_NODES] = x.astype(ml_dtypes.bfloat16)
    ones = np.ones((128, 1), ml_dtypes.bfloat16)

    in_maps = []
    for core in range(NCORES):
        idx_arr, m_arr = _build_core_arrays(es, core, tbc)
        in_maps.append({
            "x": x_pad, "idx": idx_arr, "m": m_arr,
            "W": W_np, "b": b_np, "ones": ones,
        })

    _last["nc"] = nc
    _last["in_maps"] = in_maps
    res = run_bass_kernel_spmd(nc, in_maps, list(range(NCORES)))

    full = np.concatenate([res.results[c]["out"] for c in range(NCORES)], axis=0)
    out = np.empty((N_NODES, D), np.float32)
    out[:] = full[slot_of_node]
    return out


# revision 12
# speedup vs baseline: 1.3115x; 1.0770x over previous
"""GCN layer (gather-scatter message passing) on 8 Trainium2 NeuronCores.

Strategy (dest-node sharding, per the sharding hint):
  out = segment_sum(w_e * (xW+b)[col_e] by row_e)
      = (A @ x) @ W + deg * b          with deg = A @ 1
so the big gather/scatter runs on raw x and the dense transform is applied
once per destination block afterwards.

Per core (SPMD, identical program):
  - 98 dest blocks of 128 nodes; host assigns nodes to blocks, balancing
    per-(block, source-chunk) edge counts under a hard cap.
  - x (bf16) is gathered per edge straight from DRAM via gpsimd dma_gather
    (int16 indices -> 4 source chunks of 25088 rows).
  - Per 128-edge tile: PE accumulates psum1[ch,dest] += X_t^T @ M_t and
    psum_deg[1,dest] += ones^T @ M_t, where M_t[e,d] = w_e*(dest[e]==d) is
    the host-prepared one-hot scatter operator (bf16, streamed from DRAM).
  - Per block: out_block = psum1^T @ W + deg^T @ b (fp32), DMA out.
Host gathers the 8 outputs and undoes the node permutation.
"""
import sys
sys.path.insert(0, "/opt/trn_rl_repo")
import numpy as np

N_NODES = 100000
N_EDGES = 625000
D = 128
NCORES = 8
BLK = 128
NB = 98                      # dest blocks per core
NPC = NB * BLK               # 12544 nodes per core
NBINS = NCORES * NB          # 784
NCHUNK = 4
CH_ROWS = 25088              # source chunk rows (int16-addressable)
XPAD = NCHUNK * CH_ROWS      # 100352
GB = 14                      # blocks per gather group
NG = NB // GB                # 7 groups
CAP_BASE = 2                 # tiles per (block, chunk) unless data demands more

_cache = {}
_last = {}


def _build_program(tbc):
    from concourse import bass, mybir
    import concourse.tile as tile
    from concourse import library_config

    tiles_per_block = NCHUNK * tbc
    t_total = NB * tiles_per_block
    call_idxs = GB * tbc * BLK
    idx_cols = call_idxs // 16
    ncalls = NCHUNK * NG
    slots = GB * tbc

    nc = bass.Bass(num_swdge_queues=4, dynamic_dma_scratch_size=32768)
    x_d = nc.declare_dram_parameter("x", [XPAD, D], mybir.dt.bfloat16, isOutput=False)
    idx_d = nc.declare_dram_parameter("idx", [128, ncalls * idx_cols], mybir.dt.int16, isOutput=False)
    m_d = nc.declare_dram_parameter("m", [128, t_total * BLK], mybir.dt.bfloat16, isOutput=False)
    W_d = nc.declare_dram_parameter("W", [128, 128], mybir.dt.float32, isOutput=False)
    b_d = nc.declare_dram_parameter("b", [1, 128], mybir.dt.float32, isOutput=False)
    ones_d = nc.declare_dram_parameter("ones", [128, 1], mybir.dt.bfloat16, isOutput=False)
    out_d = nc.declare_dram_parameter("out", [NPC, D], mybir.dt.float32, isOutput=True)

    with tile.TileContext(nc) as tc:
        with (
            tc.tile_pool(name="persist", bufs=1) as persist,
            tc.tile_pool(name="stage", bufs=12) as stage,
            tc.tile_pool(name="mstage", bufs=8) as mstage,
            tc.tile_pool(name="sbout", bufs=4) as sbout,
            tc.tile_pool(name="psum1", bufs=4, space="PSUM") as psum1p,
            tc.tile_pool(name="psumd", bufs=2, space="PSUM") as psumdp,
            tc.tile_pool(name="psum2", bufs=2, space="PSUM") as psum2p,
        ):
            with tc.tile_critical():
                nc.gpsimd.load_library(library_config.mlp)
            idx_t = persist.tile([128, ncalls * idx_cols], mybir.dt.int16)
            nc.sync.dma_start(out=idx_t[:], in_=idx_d[:])
            W_t = persist.tile([128, 128], mybir.dt.float32)
            nc.sync.dma_start(out=W_t[:], in_=W_d[:])
            b_t = persist.tile([1, 128], mybir.dt.float32)
            nc.sync.dma_start(out=b_t[:], in_=b_d[:])
            ones_t = persist.tile([128, 1], mybir.dt.bfloat16)
            nc.sync.dma_start(out=ones_t[:], in_=ones_d[:])

            for g in range(NG):
                stages = []
                for c in range(NCHUNK):
                    call_index = g * NCHUNK + c
                    st = stage.tile([128, slots, 128], mybir.dt.bfloat16)
                    nc.gpsimd.dma_gather(
                        out_ap=st[:],
                        in_ap=x_d[c * CH_ROWS:(c + 1) * CH_ROWS, :],
                        idxs_ap=idx_t[:, call_index * idx_cols:(call_index + 1) * idx_cols],
                        num_idxs=call_idxs,
                        num_idxs_reg=call_idxs,
                        elem_size=D,
                        single_packet=False,
                        queue_num=call_index % 4,
                    )
                    stages.append(st)
                for bloc in range(GB):
                    b = g * GB + bloc
                    mt = mstage.tile([128, tiles_per_block, BLK], mybir.dt.bfloat16)
                    nc.sync.dma_start(
                        out=mt[:],
                        in_=m_d[:, b * tiles_per_block * BLK:(b + 1) * tiles_per_block * BLK],
                    )
                    ps1 = psum1p.tile([128, 128], mybir.dt.float32, space="PSUM")
                    psd = psumdp.tile([1, 512], mybir.dt.float32, space="PSUM")
                    ti = 0
                    for c in range(NCHUNK):
                        for k in range(tbc):
                            slot = bloc * tbc + k
                            xs = stages[c][:, slot, :]
                            ms = mt[:, c * tbc + k, :]
                            first, last = ti == 0, ti == tiles_per_block - 1
                            nc.tensor.matmul(ps1[:], lhsT=xs, rhs=ms, start=first, stop=last)
                            ti += 1
                    half = tiles_per_block // 2
                    nc.tensor.matmul(psd[:], lhsT=ones_t[:],
                                     rhs=mt[:, 0:half, :].rearrange("p a b -> p (a b)"),
                                     start=True, stop=False)
                    nc.tensor.matmul(psd[:], lhsT=ones_t[:],
                                     rhs=mt[:, half:tiles_per_block, :].rearrange("p a b -> p (a b)"),
                                     start=False, stop=True)
                    axT = sbout.tile([128, 128], mybir.dt.float32)
                    nc.scalar.mul(axT[:], ps1[:], 1.0)
                    deg = sbout.tile([1, 128], mybir.dt.float32)
                    nc.vector.tensor_reduce(
                        out=deg[:], in_=psd[:].rearrange("p (a b) -> p b a", a=half),
                        axis=mybir.AxisListType.X, op=mybir.AluOpType.add)
                    ps2 = psum2p.tile([128, 128], mybir.dt.float32, space="PSUM")
                    nc.tensor.matmul(ps2[:], lhsT=axT[:], rhs=W_t[:], start=True, stop=False)
                    nc.tensor.matmul(ps2[:], lhsT=deg[:], rhs=b_t[:], start=False, stop=True)
                    ob = sbout.tile([128, 128], mybir.dt.float32)
                    nc.scalar.mul(ob[:], ps2[:], 1.0)
                    nc.sync.dma_start(out=out_d[b * BLK:(b + 1) * BLK, :], in_=ob[:])

    mybir.codegen_inst_isa_subclasses(nc)
    _fix_multiwait(nc)
    return nc


def _fix_multiwait(nc):
    """This walrus build supports ONE sync-wait per instruction; split any
    instruction carrying more onto same-engine wait-carrier nops."""
    from concourse import mybir
    ctr = 0
    for fn in nc.m.functions:
        for bb in fn.blocks:
            live = bb.instructions
            snap = list(live)
            pos = 0
            for inst in snap:
                si = inst.sync_info
                if si is not None and si.on_wait is not None and len(si.on_wait) > 1:
                    waits = list(si.on_wait)
                    si.on_wait = [waits[-1]]
                    for w in waits[:-1]:
                        n = mybir.InstNoOp(name=f"mwsplit{ctr}", ins=[], outs=[])
                        ctr += 1
                        n.engine = inst.engine
                        n.sync_info = type(si)(on_wait=[w], on_update=[])
                        live.insert(pos, n)
                        pos += 1
                pos += 1


def _prepare(x, edge_index, edge_weight):
    """Host-side sharding: node->bin assignment + per-core tile layout."""
    dest = np.asarray(edge_index[0], dtype=np.int64)
    src = np.asarray(edge_index[1], dtype=np.int64)
    w = np.asarray(edge_weight, dtype=np.float32)

    deg = np.bincount(dest, minlength=N_NODES)
    order = np.argsort(-deg, kind="stable")
    bin_of_node = np.empty(N_NODES, dtype=np.int64)
    bin_of_node[order] = np.arange(N_NODES, dtype=np.int64) % NBINS

    echunk = src // CH_ROWS
    cap = CAP_BASE * BLK

    # repair: enforce per-(bin, chunk) edge count <= cap by moving nodes
    for _ in range(200):
        ekey = bin_of_node[dest] * NCHUNK + echunk
        counts = np.bincount(ekey, minlength=NBINS * NCHUNK).reshape(NBINS, NCHUNK)
        bin_sizes = np.bincount(bin_of_node, minlength=NBINS)
        over = np.argwhere(counts > cap)
        if len(over) == 0:
            break
        bidx, c = over[0]
        nodes_in_bin = np.where(bin_of_node == bidx)[0]
        sel = (bin_of_node[dest] == bidx) & (echunk == c)
        nd = np.bincount(dest[sel], minlength=N_NODES)[nodes_in_bin]
        mover = nodes_in_bin[np.argmax(nd)]
        room = (bin_sizes < BLK)
        cand = np.where(room)[0]
        if len(cand) == 0:
            cand = np.arange(NBINS)
        tgt = cand[np.argmin(counts[cand, c])]
        bin_of_node[mover] = tgt

    ekey = bin_of_node[dest] * NCHUNK + echunk
    counts = np.bincount(ekey, minlength=NBINS * NCHUNK).reshape(NBINS, NCHUNK)
    tbc = max(CAP_BASE, int(np.ceil(counts.max() / BLK)))

    # position of each node within its block
    nodesort = np.argsort(bin_of_node, kind="stable")
    bin_sizes = np.bincount(bin_of_node, minlength=NBINS)
    starts = np.zeros(NBINS + 1, dtype=np.int64)
    np.cumsum(bin_sizes, out=starts[1:])
    pos_in_block = np.empty(N_NODES, dtype=np.int64)
    pos_in_block[nodesort] = np.arange(N_NODES) - starts[bin_of_node[nodesort]]
    assert pos_in_block.max() < BLK

    slot_of_node = bin_of_node * BLK + pos_in_block
    ebin = bin_of_node[dest]
    ekey = ebin * NCHUNK + echunk
    edge_order = np.argsort(ekey, kind="stable")
    ekey_s = ekey[edge_order]
    key_counts = np.bincount(ekey_s, minlength=NBINS * NCHUNK)
    key_starts = np.zeros(NBINS * NCHUNK + 1, dtype=np.int64)
    np.cumsum(key_counts, out=key_starts[1:])
    lane = np.arange(N_EDGES) - key_starts[ekey_s]

    es = {
        "src": src[edge_order], "w": w[edge_order],
        "dlocal": pos_in_block[dest[edge_order]],
        "bin": ebin[edge_order], "chunk": echunk[edge_order], "lane": lane,
    }
    return es, slot_of_node, tbc


def _build_core_arrays(es, core, tbc):
    import ml_dtypes
    tiles_per_block = NCHUNK * tbc
    t_total = NB * tiles_per_block
    call_idxs = GB * tbc * BLK
    idx_cols = call_idxs // 16
    ncalls = NCHUNK * NG

    sel = (es["bin"] >= core * NB) & (es["bin"] < (core + 1) * NB)
    b = es["bin"][sel] - core * NB
    c = es["chunk"][sel]
    lane = es["lane"][sel]
    srcl = (es["src"][sel] - c * CH_ROWS).astype(np.int16)
    dl = es["dlocal"][sel]
    wv = es["w"][sel]

    k = lane // BLK
    p = lane % BLK
    t_loc = b * tiles_per_block + c * tbc + k

    # host-built one-hot scatter operator: m[p, t_loc*BLK + dl] = w
    m_arr = np.zeros((128, t_total * BLK), ml_dtypes.bfloat16)
    m_arr[p, t_loc * BLK + dl] = wv

    # gather call layout: call (g, c) at call_index = g*NCHUNK+c
    g = b // GB
    call_index = g * NCHUNK + c
    i_call = (b % GB) * (tbc * BLK) + lane
    flat_idx = np.zeros(ncalls * call_idxs, np.int16)
    flat_idx[call_index * call_idxs + i_call] = srcl
    idx_arr = np.empty((128, ncalls * idx_cols), np.int16)
    wrapped = (
        flat_idx.reshape(ncalls, idx_cols, 16)
        .transpose(2, 0, 1)
        .reshape(16, ncalls * idx_cols)
    )
    for gg in range(8):
        idx_arr[gg * 16:(gg + 1) * 16, :] = wrapped
    return idx_arr, m_arr


def kernel(x, edge_index, edge_weight, W, b):
    import ml_dtypes
    from concourse.bass_utils import run_bass_kernel_spmd

    x = np.asarray(x, dtype=np.float32)
    W_np = np.asarray(W, dtype=np.float32)
    b_np = np.asarray(b, dtype=np.float32).reshape(1, D)

    es, slot_of_node, tbc = _prepare(x, edge_index, edge_weight)

    if tbc not in _cache:
        _cache[tbc] = _build_program(tbc)
    nc = _cache[tbc]

    x_pad = np.zeros((XPAD, D), ml_dtypes.bfloat16)
    x_pad[:N_NODES] = x.astype(ml_dtypes.bfloat16)
    ones = np.ones((128, 1), ml_dtypes.bfloat16)

    in_maps = []
    for core in range(NCORES):
        idx_arr, m_arr = _build_core_arrays(es, core, tbc)
        in_maps.append({
            "x": x_pad, "idx": idx_arr, "m": m_arr,
            "W": W_np, "b": b_np, "ones": ones,
        })

    _last["nc"] = nc
    _last["in_maps"] = in_maps
    res = run_bass_kernel_spmd(nc, in_maps, list(range(NCORES)))

    full = np.concatenate([res.results[c]["out"] for c in range(NCORES)], axis=0)
    out = np.empty((N_NODES, D), np.float32)
    out[:] = full[slot_of_node]
    return out


# revision 13
# speedup vs baseline: 1.3946x; 1.0634x over previous
"""GCN layer (gather-scatter message passing) on 8 Trainium2 NeuronCores.

Strategy (dest-node sharding, per the sharding hint):
  out = segment_sum(w_e * (xW+b)[col_e] by row_e)
      = (A @ x) @ W + deg * b          with deg = A @ 1
so the big gather/scatter runs on raw x and the dense transform is applied
once per destination block afterwards.

Per core (SPMD, identical program):
  - 98 dest blocks of 128 nodes; host assigns nodes to blocks, balancing
    per-(block, source-chunk) edge counts under a hard cap.
  - x (bf16) is gathered per edge straight from DRAM via gpsimd dma_gather
    (int16 indices -> 4 source chunks of 25088 rows).
  - Per 128-edge tile: PE accumulates psum1[ch,dest] += X_t^T @ M_t and
    psum_deg[1,dest] += ones^T @ M_t, where M_t[e,d] = w_e*(dest[e]==d) is
    the host-prepared one-hot scatter operator (bf16, streamed from DRAM).
  - Per block: out_block = psum1^T @ W + deg^T @ b (fp32), DMA out.
Host gathers the 8 outputs and undoes the node permutation.
"""
import sys
sys.path.insert(0, "/opt/trn_rl_repo")
import numpy as np

N_NODES = 100000
N_EDGES = 625000
D = 128
NCORES = 8
BLK = 128
NB = 98                      # dest blocks per core
NPC = NB * BLK               # 12544 nodes per core
NBINS = NCORES * NB          # 784
NCHUNK = 4
CH_ROWS = 25088              # source chunk rows (int16-addressable)
XPAD = NCHUNK * CH_ROWS      # 100352
GB = 14                      # blocks per gather group
NG = NB // GB                # 7 groups
CAP_BASE = 2                 # tiles per (block, chunk) unless data demands more

_cache = {}
_last = {}


def _build_program(tbc):
    from concourse import bass, mybir
    import concourse.tile as tile
    from concourse import library_config

    tiles_per_block = NCHUNK * tbc
    t_total = NB * tiles_per_block
    call_idxs = GB * tbc * BLK
    idx_cols = call_idxs // 16
    ncalls = NCHUNK * NG
    slots = GB * tbc

    nc = bass.Bass(num_swdge_queues=4, dynamic_dma_scratch_size=32768)
    x_d = nc.declare_dram_parameter("x", [XPAD, D], mybir.dt.bfloat16, isOutput=False)
    idx_d = nc.declare_dram_parameter("idx", [128, ncalls * idx_cols], mybir.dt.int16, isOutput=False)
    m_d = nc.declare_dram_parameter("m", [128, t_total * BLK], mybir.dt.bfloat16, isOutput=False)
    W_d = nc.declare_dram_parameter("W", [128, 128], mybir.dt.float32, isOutput=False)
    b_d = nc.declare_dram_parameter("b", [1, 128], mybir.dt.float32, isOutput=False)
    ones_d = nc.declare_dram_parameter("ones", [128, 1], mybir.dt.bfloat16, isOutput=False)
    out_d = nc.declare_dram_parameter("out", [NPC, D], mybir.dt.float32, isOutput=True)

    with tile.TileContext(nc) as tc:
        with (
            tc.tile_pool(name="persist", bufs=1) as persist,
            tc.tile_pool(name="stage", bufs=12) as stage,
            tc.tile_pool(name="mstage", bufs=8) as mstage,
            tc.tile_pool(name="sbout", bufs=4) as sbout,
            tc.tile_pool(name="psum1", bufs=4, space="PSUM") as psum1p,
            tc.tile_pool(name="psumd", bufs=2, space="PSUM") as psumdp,
            tc.tile_pool(name="psum2", bufs=2, space="PSUM") as psum2p,
        ):
            with tc.tile_critical():
                nc.gpsimd.load_library(library_config.mlp)
            idx_t = persist.tile([128, ncalls * idx_cols], mybir.dt.int16)
            nc.sync.dma_start(out=idx_t[:], in_=idx_d[:])
            W_t = persist.tile([128, 128], mybir.dt.float32)
            nc.sync.dma_start(out=W_t[:], in_=W_d[:])
            b_t = persist.tile([1, 128], mybir.dt.float32)
            nc.sync.dma_start(out=b_t[:], in_=b_d[:])
            ones_t = persist.tile([128, 1], mybir.dt.bfloat16)
            nc.sync.dma_start(out=ones_t[:], in_=ones_d[:])

            CORDER = [1, 2, 3, 0]
            for g in range(NG):
                stages = [None] * NCHUNK
                for c in CORDER:
                    call_index = g * NCHUNK + c
                    st = stage.tile([128, slots, 128], mybir.dt.bfloat16)
                    nc.gpsimd.dma_gather(
                        out_ap=st[:],
                        in_ap=x_d[c * CH_ROWS:(c + 1) * CH_ROWS, :],
                        idxs_ap=idx_t[:, call_index * idx_cols:(call_index + 1) * idx_cols],
                        num_idxs=call_idxs,
                        num_idxs_reg=call_idxs,
                        elem_size=D,
                        single_packet=False,
                        queue_num=c,
                    )
                    stages[c] = st
                for bloc in range(GB):
                    b = g * GB + bloc
                    mt = mstage.tile([128, tiles_per_block, BLK], mybir.dt.bfloat16)
                    nc.sync.dma_start(
                        out=mt[:],
                        in_=m_d[:, b * tiles_per_block * BLK:(b + 1) * tiles_per_block * BLK],
                    )
                    ps1 = psum1p.tile([128, 128], mybir.dt.float32, space="PSUM")
                    psd = psumdp.tile([1, 512], mybir.dt.float32, space="PSUM")
                    ti = 0
                    for c in CORDER:
                        for k in range(tbc):
                            slot = bloc * tbc + k
                            xs = stages[c][:, slot, :]
                            ms = mt[:, c * tbc + k, :]
                            first, last = ti == 0, ti == tiles_per_block - 1
                            nc.tensor.matmul(ps1[:], lhsT=xs, rhs=ms, start=first, stop=last)
                            ti += 1
                    half = tiles_per_block // 2
                    nc.tensor.matmul(psd[:], lhsT=ones_t[:],
                                     rhs=mt[:, 0:half, :].rearrange("p a b -> p (a b)"),
                                     start=True, stop=False)
                    nc.tensor.matmul(psd[:], lhsT=ones_t[:],
                                     rhs=mt[:, half:tiles_per_block, :].rearrange("p a b -> p (a b)"),
                                     start=False, stop=True)
                    axT = sbout.tile([128, 128], mybir.dt.float32)
                    nc.scalar.mul(axT[:], ps1[:], 1.0)
                    deg = sbout.tile([1, 128], mybir.dt.float32)
                    nc.vector.tensor_reduce(
                        out=deg[:], in_=psd[:].rearrange("p (a b) -> p b a", a=half),
                        axis=mybir.AxisListType.X, op=mybir.AluOpType.add)
                    ps2 = psum2p.tile([128, 128], mybir.dt.float32, space="PSUM")
                    nc.tensor.matmul(ps2[:], lhsT=axT[:], rhs=W_t[:], start=True, stop=False)
                    nc.tensor.matmul(ps2[:], lhsT=deg[:], rhs=b_t[:], start=False, stop=True)
                    ob = sbout.tile([128, 128], mybir.dt.float32)
                    nc.scalar.mul(ob[:], ps2[:], 1.0)
                    nc.sync.dma_start(out=out_d[b * BLK:(b + 1) * BLK, :], in_=ob[:])

    mybir.codegen_inst_isa_subclasses(nc)
    _fix_multiwait(nc)
    return nc


def _fix_multiwait(nc):
    """This walrus build supports ONE sync-wait per instruction; split any
    instruction carrying more onto same-engine wait-carrier nops."""
    from concourse import mybir
    ctr = 0
    for fn in nc.m.functions:
        for bb in fn.blocks:
            live = bb.instructions
            snap = list(live)
            pos = 0
            for inst in snap:
                si = inst.sync_info
                if si is not None and si.on_wait is not None and len(si.on_wait) > 1:
                    waits = list(si.on_wait)
                    si.on_wait = [waits[-1]]
                    for w in waits[:-1]:
                        n = mybir.InstNoOp(name=f"mwsplit{ctr}", ins=[], outs=[])
                        ctr += 1
                        n.engine = inst.engine
                        n.sync_info = type(si)(on_wait=[w], on_update=[])
                        live.insert(pos, n)
                        pos += 1
                pos += 1


def _prepare(x, edge_index, edge_weight):
    """Host-side sharding: node->bin assignment + per-core tile layout."""
    dest = np.asarray(edge_index[0], dtype=np.int64)
    src = np.asarray(edge_index[1], dtype=np.int64)
    w = np.asarray(edge_weight, dtype=np.float32)

    deg = np.bincount(dest, minlength=N_NODES)
    order = np.argsort(-deg, kind="stable")
    bin_of_node = np.empty(N_NODES, dtype=np.int64)
    bin_of_node[order] = np.arange(N_NODES, dtype=np.int64) % NBINS

    echunk = src // CH_ROWS
    cap = CAP_BASE * BLK

    # repair: enforce per-(bin, chunk) edge count <= cap by moving nodes
    for _ in range(200):
        ekey = bin_of_node[dest] * NCHUNK + echunk
        counts = np.bincount(ekey, minlength=NBINS * NCHUNK).reshape(NBINS, NCHUNK)
        bin_sizes = np.bincount(bin_of_node, minlength=NBINS)
        over = np.argwhere(counts > cap)
        if len(over) == 0:
            break
        bidx, c = over[0]
        nodes_in_bin = np.where(bin_of_node == bidx)[0]
        sel = (bin_of_node[dest] == bidx) & (echunk == c)
        nd = np.bincount(dest[sel], minlength=N_NODES)[nodes_in_bin]
        mover = nodes_in_bin[np.argmax(nd)]
        room = (bin_sizes < BLK)
        cand = np.where(room)[0]
        if len(cand) == 0:
            cand = np.arange(NBINS)
        tgt = cand[np.argmin(counts[cand, c])]
        bin_of_node[mover] = tgt

    ekey = bin_of_node[dest] * NCHUNK + echunk
    counts = np.bincount(ekey, minlength=NBINS * NCHUNK).reshape(NBINS, NCHUNK)
    tbc = max(CAP_BASE, int(np.ceil(counts.max() / BLK)))

    # position of each node within its block
    nodesort = np.argsort(bin_of_node, kind="stable")
    bin_sizes = np.bincount(bin_of_node, minlength=NBINS)
    starts = np.zeros(NBINS + 1, dtype=np.int64)
    np.cumsum(bin_sizes, out=starts[1:])
    pos_in_block = np.empty(N_NODES, dtype=np.int64)
    pos_in_block[nodesort] = np.arange(N_NODES) - starts[bin_of_node[nodesort]]
    assert pos_in_block.max() < BLK

    slot_of_node = bin_of_node * BLK + pos_in_block
    ebin = bin_of_node[dest]
    ekey = ebin * NCHUNK + echunk
    edge_order = np.argsort(ekey, kind="stable")
    ekey_s = ekey[edge_order]
    key_counts = np.bincount(ekey_s, minlength=NBINS * NCHUNK)
    key_starts = np.zeros(NBINS * NCHUNK + 1, dtype=np.int64)
    np.cumsum(key_counts, out=key_starts[1:])
    lane = np.arange(N_EDGES) - key_starts[ekey_s]

    es = {
        "src": src[edge_order], "w": w[edge_order],
        "dlocal": pos_in_block[dest[edge_order]],
        "bin": ebin[edge_order], "chunk": echunk[edge_order], "lane": lane,
    }
    return es, slot_of_node, tbc


def _build_core_arrays(es, core, tbc):
    import ml_dtypes
    tiles_per_block = NCHUNK * tbc
    t_total = NB * tiles_per_block
    call_idxs = GB * tbc * BLK
    idx_cols = call_idxs // 16
    ncalls = NCHUNK * NG

    sel = (es["bin"] >= core * NB) & (es["bin"] < (core + 1) * NB)
    b = es["bin"][sel] - core * NB
    c = es["chunk"][sel]
    lane = es["lane"][sel]
    srcl = (es["src"][sel] - c * CH_ROWS).astype(np.int16)
    dl = es["dlocal"][sel]
    wv = es["w"][sel]

    k = lane // BLK
    p = lane % BLK
    t_loc = b * tiles_per_block + c * tbc + k

    # host-built one-hot scatter operator: m[p, t_loc*BLK + dl] = w
    m_arr = np.zeros((128, t_total * BLK), ml_dtypes.bfloat16)
    m_arr[p, t_loc * BLK + dl] = wv

    # gather call layout: call (g, c) at call_index = g*NCHUNK+c
    g = b // GB
    call_index = g * NCHUNK + c
    i_call = (b % GB) * (tbc * BLK) + lane
    flat_idx = np.zeros(ncalls * call_idxs, np.int16)
    flat_idx[call_index * call_idxs + i_call] = srcl
    idx_arr = np.empty((128, ncalls * idx_cols), np.int16)
    wrapped = (
        flat_idx.reshape(ncalls, idx_cols, 16)
        .transpose(2, 0, 1)
        .reshape(16, ncalls * idx_cols)
    )
    for gg in range(8):
        idx_arr[gg * 16:(gg + 1) * 16, :] = wrapped
    return idx_arr, m_arr


def kernel(x, edge_index, edge_weight, W, b):
    import ml_dtypes
    from concourse.bass_utils import run_bass_kernel_spmd

    x = np.asarray(x, dtype=np.float32)
    W_np = np.asarray(W, dtype=np.float32)
    b_np = np.asarray(b, dtype=np.float32).reshape(1, D)

    es, slot_of_node, tbc = _prepare(x, edge_index, edge_weight)

    if tbc not in _cache:
        _cache[tbc] = _build_program(tbc)
    nc = _cache[tbc]

    x_pad = np.zeros((XPAD, D), ml_dtypes.bfloat16)
    x_pad[:N_NODES] = x.astype(ml_dtypes.bfloat16)
    ones = np.ones((128, 1), ml_dtypes.bfloat16)

    in_maps = []
    for core in range(NCORES):
        idx_arr, m_arr = _build_core_arrays(es, core, tbc)
        in_maps.append({
            "x": x_pad, "idx": idx_arr, "m": m_arr,
            "W": W_np, "b": b_np, "ones": ones,
        })

    _last["nc"] = nc
    _last["in_maps"] = in_maps
    res = run_bass_kernel_spmd(nc, in_maps, list(range(NCORES)))

    full = np.concatenate([res.results[c]["out"] for c in range(NCORES)], axis=0)
    out = np.empty((N_NODES, D), np.float32)
    out[:] = full[slot_of_node]
    return out
